# revision 41
# baseline (speedup 1.0000x reference)
"""CondTransport kernel for 8x Trainium2 NeuronCores (v2, pipelined).

Math (per reference):
  x_mean = [x_mu, y_mean+y_var]                      [Nq, 64]
  x_var  = [x_mu, 0.01*flip(y_eta), y_mean+y_var]    [Nq, 96]
  Lam_m  = kXXmean_inv @ Z_mean                      [Nx, 32]
  Lam_v  = kXXvar_inv  @ Z_var                       [Nx, 32]
  K_m    = exp(-d2(X_mean, x_mean)/128);  z_m = K_m.T @ Lam_m
  K_v    = exp(-d2(X_var,  x_var )/128);  z_v = K_v.T @ Lam_v
  out    = y_mean + y_var + z_m + z_v                [Nq, 32]

Design notes (v2):
  - Queries sharded across 8 cores (1024 each); Lambda row-sharded with the
    row block split into S_SPLIT=4 sub-blocks, each AllGathered separately so
    Lambda tiles become available progressively while the RBF phases run.
  - All large matmul operands in bf16 (inv Grams cast on host): halves the
    dominant HBM stream and enables fast weight loads.
  - RBF factorization exp(-d2/128) = exp((S - |X|^2/2)/64) * exp(-|xq|^2/128)
    with the -|X|^2/2 term supplied as an EXTRA CONTRACTION ROW of the
    stationary operand (moving operand carries a ones-row), so the exp can
    run over [128, 4096] slabs with a single scale and no per-tile bias.
  - S tiles are DVE-copied from PSUM into [128, 4096] f32 slabs; one ACT exp
    per slab (amortizes the ~352-cycle ACT instruction overhead).
  - z and stage-A matmuls have 32-wide outputs: 4 are packed into the PE
    array concurrently via column tiling (tile_position), with a cross-
    partition DVE add at the end.
"""
import os
import sys

sys.path.insert(0, "/opt/trn_rl_repo")

import numpy as np
import ml_dtypes
from contextlib import ExitStack

import concourse.bacc as bacc
import concourse.masks as masks
import concourse.mybir as mybir
import concourse.tile as tile
from concourse.bass_utils import run_bass_kernel_spmd

NX = 8192
NQ = 8192
DX = 32
DY = 32
DM = 64          # x_mean feature dim
DV = 96          # x_var feature dim
NCORES = 8
QLOC = NQ // NCORES           # 1024 queries per core
RLOC = NX // NCORES           # 1024 Lambda rows per core
NXT = NX // 128               # 64 x-tiles
QT = QLOC // 128              # 8 local q-tiles

S_SPLIT = 2                   # Lambda sub-gathers per matrix
RSUB = RLOC // S_SPLIT        # 256 Lambda rows per core per sub-gather
ISUB = RSUB // 128            # 2 j-tiles contributed per core per sub-gather
NCH = 16                      # inv DMA chunks per sub-block
KTC = NXT // NCH              # 4 k-tiles per chunk
NST = 16                      # exp stage-tiles per phase (4 j-tiles each)
JPS = 4                       # j-tiles per stage
STW = JPS * QLOC              # stage width: 4096

F32 = mybir.dt.float32
BF16 = mybir.dt.bfloat16
EXP = mybir.ActivationFunctionType.Exp

_CACHED_NC = None

KT_BUFS = int(os.environ.get("CTK_KT_BUFS", "28"))
Z_LAG = int(os.environ.get("CTK_ZLAG", "2"))


def _jlist(phase_s0):
    """Phase j-tile consumption order: gather-availability order.

    Sub-gather s of this matrix yields j-tiles {8c + ISUB*s + i}.
    """
    out = []
    for s in range(S_SPLIT):
        for c in range(NCORES):
            for i in range(ISUB):
                out.append(8 * c + ISUB * s + i)
    return out


def _build_nc():
    nc = bacc.Bacc("TRN2", target_bir_lowering=False, debug=False,
                   num_devices=NCORES)

    # ---------------- I/O ----------------
    def inp(name, shape, dt=BF16):
        return nc.dram_tensor(name, list(shape), dt, kind="ExternalInput").ap()

    invm = inp("invm", (S_SPLIT, NCH, 128, KTC * RSUB))   # packed invT slabs
    invv = inp("invv", (S_SPLIT, NCH, 128, KTC * RSUB))
    XmT = inp("XmT", (DM, NX))            # X_mean.T
    XvT = inp("XvT", (DV, NX))            # X_var.T
    Zm = inp("Zm", (128, NXT * DY))       # packed (p, kt, d)
    Zv = inp("Zv", (128, NXT * DY))
    xmuT = inp("xmuT", (DX, QLOC))        # x_mu.T slice
    yefT = inp("yefT", (DY, QLOC))        # flip(y_eta).T slice (unscaled)
    ymT = inp("ymT", (DY, QLOC))
    yvT = inp("yvT", (DY, QLOC))
    qpk = inp("qpk", (128, 4 * QT * DY), F32)   # packed naturals (t, jq, d)

    out = nc.dram_tensor("out", [QLOC, DY], F32, kind="ExternalOutput").ap()

    # collective bounce buffers (per matrix x sub-block)
    lam_in = {}
    lam_out = {}
    for mat in "mv":
        for s in range(S_SPLIT):
            lam_in[mat, s] = nc.dram_tensor(
                f"lam_in_{mat}{s}", [RSUB, DY], F32, kind="Internal").ap()
            lam_out[mat, s] = nc.dram_tensor(
                f"lam_out_{mat}{s}", [NCORES * RSUB, DY], F32,
                kind="Internal", addr_space="Shared").ap()

    with tile.TileContext(nc) as tc, ExitStack() as ctx:
        P = lambda **kw: ctx.enter_context(tc.tile_pool(**kw))
        const = P(name="const", bufs=1)
        ktp = P(name="ktp", bufs=KT_BUFS)   # [128, 1024] bf16 exp tiles
        invp = P(name="invp", bufs=3)       # inv chunks
        lstp = P(name="lstp", bufs=2)       # lambda gather stage-in
        work = P(name="work", bufs=1)
        psS = P(name="psS", bufs=2, space="PSUM")    # S matmul 2-bank pairs
        psZ = P(name="psZ", bufs=1, space="PSUM")    # z accumulators
        psA = P(name="psA", bufs=1, space="PSUM")    # stage-A accumulator
        psT = P(name="psT", bufs=1, space="PSUM")    # transposes

        ident = const.tile([128, 128], F32, tag="ident")
        masks.make_identity(nc, ident[:])

        # ------- setup loads: X-norm chain first (it gates all S matmuls) ---
        XmT_sb = const.tile([DM + 1, NX], BF16, tag="XmT_sb")
        nc.sync.dma_start(XmT_sb[0:DM, :], XmT)
        ones_sb = const.tile([128, 1], BF16, tag="ones_sb")
        nc.vector.memset(ones_sb[:], 1.0)

        # --------- X norm rows (-|X|^2/2 into XT_sb row DM/DV) --------------
        # Square X.T on DVE, then a ones-vector matmul reduces over the
        # feature partitions; tile_position lands the [1, 512] result rows
        # directly on the XT norm-row partition (DM=64 / DV=96), so a plain
        # same-base copy finishes the job.
        def x_norm_row(XT_sb, dfeat, use_act):
            sq = work.tile([DV, NX], BF16, tag="xsq", name=f"xsq{dfeat}")
            nc.vector.tensor_mul(sq[0:dfeat, :], XT_sb[0:dfeat, :],
                                 XT_sb[0:dfeat, :])
            for ch in range(NX // 512):
                ps = psS.tile([128, 1024], F32, tag="ps", name=f"xn{dfeat}_{ch}")
                nc.tensor.matmul(
                    ps[dfeat:dfeat + 1, 0:512],
                    ones_sb[0:dfeat, :],
                    sq[0:dfeat, ch * 512:(ch + 1) * 512],
                    start=True, stop=True,
                    tile_position=(0, dfeat))
                dst = XT_sb[dfeat:dfeat + 1, ch * 512:(ch + 1) * 512]
                if use_act:
                    nc.scalar.mul(dst, ps[dfeat:dfeat + 1, 0:512], -0.5)
                else:
                    nc.vector.tensor_scalar_mul(
                        dst, ps[dfeat:dfeat + 1, 0:512], -0.5)

        x_norm_row(XmT_sb, DM, True)

        Zm_sb = const.tile([128, NXT * DY], BF16, tag="Zm_sb")
        nc.sync.dma_start(Zm_sb[:], Zm)

        # ---------------- query-side assembly ----------------
        # DVE tensor ops need all operands at the same start partition, so
        # stage each transposed query block at its destination partition.
        xmT = const.tile([DM + 1, QLOC], BF16, tag="xmT")
        xvT = const.tile([DV + 1, QLOC], BF16, tag="xvT")
        ym_st = const.tile([DM, QLOC], BF16, tag="ym_st")
        yv_st = const.tile([DM, QLOC], BF16, tag="yv_st")
        yef_st = const.tile([DM, QLOC], BF16, tag="yef_st")
        nc.sync.dma_start(xmT[0:DX, :], xmuT)
        nc.sync.dma_start(xvT[0:DX, :], xmuT)
        nc.sync.dma_start(ym_st[DX:DM, :], ymT)
        nc.sync.dma_start(yv_st[DX:DM, :], yvT)
        nc.sync.dma_start(yef_st[DX:DM, :], yefT)
        nc.vector.tensor_add(xmT[DX:DM, :], ym_st[DX:DM, :], yv_st[DX:DM, :])
        nc.vector.memset(xmT[DM:DM + 1, :], 1.0)
        nc.vector.tensor_scalar_mul(xvT[DX:DM, :], yef_st[DX:DM, :], 0.01)
        # realign (ym+yv).T to partitions 64:96 via SBUF->SBUF DMA
        nc.gpsimd.dma_start(xvT[DM:DV, :], xmT[DX:DM, :])
        nc.vector.memset(xvT[DV:DV + 1, :], 1.0)

        # naturals: ymv (final additive term) + query norm scales
        # (deferred: only needed by the phase tails, emitted mid-schedule)
        qpk_sb = const.tile([128, 4 * QT * DY], F32, tag="qpk_sb")
        ymv_sb = const.tile([128, QT * DY], F32, tag="ymv_sb")
        e_m = const.tile([128, QT], F32, tag="e_m")
        e_v = const.tile([128, QT], F32, tag="e_v")

        def q_norm_setup():
            nc.sync.dma_start(qpk_sb[:], qpk)
            nc.vector.tensor_add(ymv_sb[:], qpk_sb[:, 2 * QT * DY:3 * QT * DY],
                             qpk_sb[:, 3 * QT * DY:4 * QT * DY])
            qsq = work.tile([128, 2 * QT * DY], F32, tag="qsq")
            nc.vector.tensor_mul(qsq[:, 0:QT * DY], qpk_sb[:, 0:QT * DY],
                                 qpk_sb[:, 0:QT * DY])          # x_mu^2
            nc.vector.tensor_mul(qsq[:, QT * DY:],
                                 qpk_sb[:, QT * DY:2 * QT * DY],
                                 qpk_sb[:, QT * DY:2 * QT * DY])  # yef^2
            ymvsq = work.tile([128, QT * DY], F32, tag="ymvsq")
            nc.vector.tensor_mul(ymvsq[:], ymv_sb[:], ymv_sb[:])
            r_xmu = work.tile([128, QT], F32, tag="r_xmu")
            r_yef = work.tile([128, QT], F32, tag="r_yef")
            r_ymv = work.tile([128, QT], F32, tag="r_ymv")
            RD = lambda dst, src: nc.vector.tensor_reduce(
                dst[:], src.rearrange("p (j d) -> p j d", d=DY),
                mybir.AxisListType.X, mybir.AluOpType.add)
            RD(r_xmu, qsq[:, 0:QT * DY])
            RD(r_yef, qsq[:, QT * DY:])
            RD(r_ymv, ymvsq[:])
            nc.vector.tensor_add(e_m[:], r_xmu[:], r_ymv[:])
            nc.vector.tensor_scalar_mul(r_yef[:], r_yef[:], 1.0e-4)
            nc.vector.tensor_add(e_v[:], e_m[:], r_yef[:])
            nc.scalar.activation(e_m[:], e_m[:], EXP, scale=-1.0 / 128.0)
            nc.scalar.activation(e_v[:], e_v[:], EXP, scale=-1.0 / 128.0)

        # ---------------- stage A: Lambda sub-blocks + sub-gathers ----------
        def stage_a_sub(inv_dram, Z_sb, mat, s):
            pa = psA.tile([128, RSUB], F32, tag="pa", name=f"pa_{mat}{s}")
            for q in range(NCH):
                chunk = invp.tile([128, KTC * RSUB], BF16, tag="invchunk")
                nc.sync.dma_start(chunk[:], inv_dram[s, q])
                for k8 in range(KTC):
                    kt = q * KTC + k8
                    g = kt % 4
                    nc.tensor.matmul(
                        pa[32 * g:32 * (g + 1), :],
                        Z_sb[:, kt * DY:(kt + 1) * DY],
                        chunk[:, k8 * RSUB:(k8 + 1) * RSUB],
                        start=(kt < 4), stop=(kt >= NXT - 4),
                        tile_position=(0, 32 * g))
            # transpose col-group partials so the group sum becomes a legal
            # same-base free-dim add: pa [4g*32d, r] -> T [r, 4g*32d]
            paS = work.tile([128, RSUB], F32, tag="paS")
            nc.vector.tensor_copy(paS[:], pa[:])
            lam_nat = work.tile([128, ISUB * DY], F32, tag="lam_nat")
            for i in range(ISUB):
                pt = psT.tile([128, 128], F32, tag="pt")
                nc.tensor.transpose(pt[:], paS[:, i * 128:(i + 1) * 128],
                                    ident[:])
                t0 = work.tile([128, 2 * DY], F32, tag="lam_t0")
                nc.vector.tensor_copy(t0[:], pt[:, 0:64])
                nc.vector.tensor_add(t0[:], t0[:], pt[:, 64:128])
                nc.vector.tensor_add(lam_nat[:, i * DY:(i + 1) * DY],
                                     t0[:, 0:DY], t0[:, DY:2 * DY])
            nc.gpsimd.dma_start(
                lam_in[mat, s].rearrange("(i p) d -> p i d", p=128),
                lam_nat[:].rearrange("p (i d) -> p i d", d=DY))
            nc.gpsimd.collective_compute(
                "AllGather", mybir.AluOpType.bypass,
                replica_groups=[list(range(NCORES))],
                ins=[lam_in[mat, s].opt()], outs=[lam_out[mat, s].opt()])

        def lam_stage_in(lam_sb, mat, s):
            lst = lstp.tile([128, NCORES * ISUB * DY], F32, tag="lst")
            nc.gpsimd.dma_start(
                lst[:].rearrange("p (c i d) -> p c i d", c=NCORES, i=ISUB),
                lam_out[mat, s].rearrange("(c i p) d -> p c i d",
                                          i=ISUB, p=128))
            # scatter to lam_sb columns: j = 8c + ISUB*s + i
            dst = lam_sb[:].rearrange("p (c r) -> p c r", c=NCORES)[
                :, :, ISUB * DY * s:ISUB * DY * (s + 1)]
            nc.vector.tensor_copy(
                dst, lst[:].rearrange("p (c r) -> p c r", c=NCORES))

        lam_m_sb = const.tile([128, NXT * DY], BF16, tag="lam_m_sb")
        lam_v_sb = const.tile([128, NXT * DY], BF16, tag="lam_v_sb")
        XvT_sb = const.tile([DV + 1, NX], BF16, tag="XvT_sb")
        Zv_sb = const.tile([128, NXT * DY], BF16, tag="Zv_sb")

        # ---------------- RBF phase machinery ----------------
        def phase_mk(XT_sb, dfeat, xT_sb, lam_sb, combine, name):
            """Returns (emit_stage(st), emit_z(st), tail()) closures."""
            jl = _jlist(0)
            kts = {}
            pz = [psZ.tile([128, 512], F32, tag=f"pz{qc}", name=f"pz{qc}_{name}")
                  for qc in range(2)]

            def emit_stage(st):
                jset = jl[JPS * st:JPS * (st + 1)]
                for jj, j in enumerate(jset):
                    ps = psS.tile([128, 1024], F32, tag="ps")
                    for qc in range(2):
                        nc.tensor.matmul(
                            ps[:, qc * 512:(qc + 1) * 512],
                            XT_sb[0:dfeat + 1, j * 128:(j + 1) * 128],
                            xT_sb[0:dfeat + 1, qc * 512:(qc + 1) * 512],
                            start=True, stop=True)
                    kt = ktp.tile([128, QLOC], BF16, tag="kt")
                    nc.scalar.activation(kt[:], ps[:], EXP, scale=1.0 / 64.0)
                    kts[st, jj] = kt

            def emit_z(st):
                jset = jl[JPS * st:JPS * (st + 1)]
                for qc in range(2):
                    for g, j in enumerate(jset):
                        nc.tensor.matmul(
                            pz[qc][32 * g:32 * (g + 1), :],
                            lam_sb[:, j * DY:(j + 1) * DY],
                            kts[st, g][:, qc * 512:(qc + 1) * 512],
                            start=(st == 0), stop=(st == NST - 1),
                            tile_position=(0, 32 * g))

            def tail():
                # pz [4g*32d, q] -> transpose 128-col slices -> [q, 4g*32d],
                # then sum groups along the free dim (natural q layout).
                for qc in range(2):
                    pzS = work.tile([128, 512], F32, tag="pzS")
                    nc.vector.tensor_copy(pzS[:], pz[qc][:])
                    for qq in range(4):
                        jq = 4 * qc + qq
                        pt = psT.tile([128, 128], F32, tag="pt")
                        nc.tensor.transpose(
                            pt[:], pzS[:, qq * 128:(qq + 1) * 128], ident[:])
                        t0 = work.tile([128, 2 * DY], F32, tag="z_t0")
                        nc.vector.tensor_copy(t0[:], pt[:, 0:64])
                        nc.vector.tensor_add(t0[:], t0[:], pt[:, 64:128])
                        combine(jq, t0[:, 0:DY], t0[:, DY:2 * DY])

            return emit_stage, emit_z, tail

        zM_nat = const.tile([128, QT * DY], F32, tag="zM_nat")

        def combine_mean(jq, t0, t1):
            sl = slice(jq * DY, (jq + 1) * DY)
            nc.vector.tensor_add(zM_nat[:, sl], t0, t1)
            nc.vector.tensor_scalar_mul(zM_nat[:, sl], zM_nat[:, sl],
                                        e_m[:, jq:jq + 1])

        out_sb = const.tile([128, QT * DY], F32, tag="out_sb")

        def combine_var(jq, t0, t1):
            sl = slice(jq * DY, (jq + 1) * DY)
            nc.vector.tensor_add(out_sb[:, sl], t0, t1)
            nc.vector.tensor_scalar_mul(out_sb[:, sl], out_sb[:, sl],
                                        e_v[:, jq:jq + 1])
            nc.vector.tensor_add(out_sb[:, sl], out_sb[:, sl], zM_nat[:, sl])
            nc.vector.tensor_add(out_sb[:, sl], out_sb[:, sl], ymv_sb[:, sl])

        ph_m = phase_mk(XmT_sb, DM, xmT, lam_m_sb, combine_mean, "m")
        ph_v = phase_mk(XvT_sb, DV, xvT, lam_v_sb, combine_var, "v")

        # ---------------- emission schedule ----------------
        def phase_emit(ph, st):
            emit_stage, emit_z, _ = ph
            emit_stage(st)
            if st - Z_LAG >= 0:
                emit_z(st - Z_LAG)

        stage_a_sub(invm, Zm_sb, "m", 0)
        lam_stage_in(lam_m_sb, "m", 0)
        for st in range(0, 4):
            phase_emit(ph_m, st)
        stage_a_sub(invm, Zm_sb, "m", 1)
        lam_stage_in(lam_m_sb, "m", 1)
        for st in range(4, 8):
            phase_emit(ph_m, st)

        # var-side loads + deferred query-norm setup
        nc.sync.dma_start(XvT_sb[0:DV, :], XvT)
        nc.sync.dma_start(Zv_sb[:], Zv)
        q_norm_setup()

        stage_a_sub(invv, Zv_sb, "v", 0)
        lam_stage_in(lam_v_sb, "v", 0)
        for st in range(8, 12):
            phase_emit(ph_m, st)
        x_norm_row(XvT_sb, DV, False)
        for st in range(12, 16):
            phase_emit(ph_m, st)
        stage_a_sub(invv, Zv_sb, "v", 1)
        lam_stage_in(lam_v_sb, "v", 1)
        # trailing z of phase m
        for st in range(NST - Z_LAG, NST):
            ph_m[1](st)
        ph_m[2]()  # tail m

        for st in range(0, 8):
            phase_emit(ph_v, st)
        for st in range(8, 16):
            phase_emit(ph_v, st)
        for st in range(NST - Z_LAG, NST):
            ph_v[1](st)
        ph_v[2]()  # tail v (writes out_sb)

        nc.gpsimd.dma_start(out.rearrange("(t p) d -> p t d", p=128),
                            out_sb[:].rearrange("p (t d) -> p t d", d=DY))

    nc.compile()
    return nc


def get_nc():
    global _CACHED_NC
    if _CACHED_NC is None:
        _CACHED_NC = _build_nc()
    return _CACHED_NC


def _host_prep(x_mu, y_eta, y_mean, y_var, X_mean, X_var, Z_mean, Z_var,
               kXXmean_inv, kXXvar_inv):
    """Host prep: transposes / slicing / packing / bf16 casts only."""
    BF = ml_dtypes.bfloat16
    C = np.ascontiguousarray

    def pack_inv(inv):
        # per-core [S_SPLIT, NCH, 128, KTC*RSUB] slabs of inv.T in bf16
        invT = C(inv.T).astype(BF)
        packs = []
        for c in range(NCORES):
            A = invT[:, c * RLOC:(c + 1) * RLOC]
            A = A.reshape(NXT, 128, S_SPLIT, RSUB).transpose(2, 0, 1, 3)
            A = A.reshape(S_SPLIT, NCH, KTC, 128, RSUB).transpose(0, 1, 3, 2, 4)
            packs.append(C(A.reshape(S_SPLIT, NCH, 128, KTC * RSUB)))
        return packs

    invm_p = pack_inv(kXXmean_inv)
    invv_p = pack_inv(kXXvar_inv)

    XmT = C(X_mean.T).astype(BF)
    XvT = C(X_var.T).astype(BF)
    Zm = C(Z_mean.reshape(NXT, 128, DY).transpose(1, 0, 2).reshape(128, -1)
           ).astype(BF)
    Zv = C(Z_var.reshape(NXT, 128, DY).transpose(1, 0, 2).reshape(128, -1)
           ).astype(BF)
    yef = y_eta[::-1]

    in_maps = []
    for c in range(NCORES):
        q = slice(c * QLOC, (c + 1) * QLOC)
        qpk = np.stack([x_mu[q], yef[q], y_mean[q], y_var[q]])  # [4,1024,32]
        qpk = qpk.reshape(4, QT, 128, DY).transpose(2, 0, 1, 3)
        in_maps.append({
            "invm": invm_p[c], "invv": invv_p[c],
            "XmT": XmT, "XvT": XvT, "Zm": Zm, "Zv": Zv,
            "xmuT": C(x_mu[q].T).astype(BF),
            "yefT": C(yef[q].T).astype(BF),
            "ymT": C(y_mean[q].T).astype(BF),
            "yvT": C(y_var[q].T).astype(BF),
            "qpk": C(qpk.reshape(128, -1)).astype(np.float32),
        })
    return in_maps


def kernel(x_mu, y_eta, y_mean, y_var, X_mean, X_var, Z_mean, Z_var,
           kXXmean_inv, kXXvar_inv, _trace=False, _tmpdir=None):
    nc = get_nc()
    in_maps = _host_prep(x_mu, y_eta, y_mean, y_var, X_mean, X_var,
                         Z_mean, Z_var, kXXmean_inv, kXXvar_inv)
    res = run_bass_kernel_spmd(nc, in_maps, core_ids=list(range(NCORES)),
                               trace=_trace, tmpdir=_tmpdir)
    out = np.concatenate([res.results[c]["out"] for c in range(NCORES)], axis=0)
    if _trace:
        kernel._last_results = res
    return out


# revision 43
# speedup vs baseline: 1.0594x; 1.0594x over previous
"""CondTransport kernel for 8x Trainium2 NeuronCores (v2, pipelined).

Math (per reference):
  x_mean = [x_mu, y_mean+y_var]                      [Nq, 64]
  x_var  = [x_mu, 0.01*flip(y_eta), y_mean+y_var]    [Nq, 96]
  Lam_m  = kXXmean_inv @ Z_mean                      [Nx, 32]
  Lam_v  = kXXvar_inv  @ Z_var                       [Nx, 32]
  K_m    = exp(-d2(X_mean, x_mean)/128);  z_m = K_m.T @ Lam_m
  K_v    = exp(-d2(X_var,  x_var )/128);  z_v = K_v.T @ Lam_v
  out    = y_mean + y_var + z_m + z_v                [Nq, 32]

Design notes (v2):
  - Queries sharded across 8 cores (1024 each); Lambda row-sharded with the
    row block split into S_SPLIT=4 sub-blocks, each AllGathered separately so
    Lambda tiles become available progressively while the RBF phases run.
  - All large matmul operands in bf16 (inv Grams cast on host): halves the
    dominant HBM stream and enables fast weight loads.
  - RBF factorization exp(-d2/128) = exp((S - |X|^2/2)/64) * exp(-|xq|^2/128)
    with the -|X|^2/2 term supplied as an EXTRA CONTRACTION ROW of the
    stationary operand (moving operand carries a ones-row), so the exp can
    run over [128, 4096] slabs with a single scale and no per-tile bias.
  - S tiles are DVE-copied from PSUM into [128, 4096] f32 slabs; one ACT exp
    per slab (amortizes the ~352-cycle ACT instruction overhead).
  - z and stage-A matmuls have 32-wide outputs: 4 are packed into the PE
    array concurrently via column tiling (tile_position), with a cross-
    partition DVE add at the end.
"""
import os
import sys

sys.path.insert(0, "/opt/trn_rl_repo")

import numpy as np
import ml_dtypes
from contextlib import ExitStack

import concourse.bacc as bacc
import concourse.masks as masks
import concourse.mybir as mybir
import concourse.tile as tile
from concourse.bass_utils import run_bass_kernel_spmd

NX = 8192
NQ = 8192
DX = 32
DY = 32
DM = 64          # x_mean feature dim
DV = 96          # x_var feature dim
NCORES = 8
QLOC = NQ // NCORES           # 1024 queries per core
RLOC = NX // NCORES           # 1024 Lambda rows per core
NXT = NX // 128               # 64 x-tiles
QT = QLOC // 128              # 8 local q-tiles

S_SPLIT = 2                   # Lambda sub-gathers per matrix
RSUB = RLOC // S_SPLIT        # 256 Lambda rows per core per sub-gather
ISUB = RSUB // 128            # 2 j-tiles contributed per core per sub-gather
NCH = 16                      # inv DMA chunks per sub-block
KTC = NXT // NCH              # 4 k-tiles per chunk
NST = 16                      # exp stage-tiles per phase (4 j-tiles each)
JPS = 4                       # j-tiles per stage
STW = JPS * QLOC              # stage width: 4096

F32 = mybir.dt.float32
BF16 = mybir.dt.bfloat16
EXP = mybir.ActivationFunctionType.Exp

_CACHED_NC = None

KT_BUFS = int(os.environ.get("CTK_KT_BUFS", "40"))
Z_LAG = int(os.environ.get("CTK_ZLAG", "2"))


def _jlist(phase_s0):
    """Phase j-tile consumption order: gather-availability order.

    Sub-gather s of this matrix yields j-tiles {8c + ISUB*s + i}.
    """
    out = []
    for s in range(S_SPLIT):
        for c in range(NCORES):
            for i in range(ISUB):
                out.append(8 * c + ISUB * s + i)
    return out


def _build_nc():
    nc = bacc.Bacc("TRN2", target_bir_lowering=False, debug=False,
                   num_devices=NCORES)

    # ---------------- I/O ----------------
    def inp(name, shape, dt=BF16):
        return nc.dram_tensor(name, list(shape), dt, kind="ExternalInput").ap()

    invm = inp("invm", (S_SPLIT, NCH, 128, KTC * RSUB))   # packed invT slabs
    invv = inp("invv", (S_SPLIT, NCH, 128, KTC * RSUB))
    XmT = inp("XmT", (DM, NX))            # X_mean.T
    XvT = inp("XvT", (DV, NX))            # X_var.T
    Zm = inp("Zm", (128, NXT * DY))       # packed (p, kt, d)
    Zv = inp("Zv", (128, NXT * DY))
    xmuT = inp("xmuT", (DX, QLOC))        # x_mu.T slice
    yefT = inp("yefT", (DY, QLOC))        # flip(y_eta).T slice (unscaled)
    ymT = inp("ymT", (DY, QLOC))
    yvT = inp("yvT", (DY, QLOC))
    qpk = inp("qpk", (128, 4 * QT * DY), F32)   # packed naturals (t, jq, d)

    out = nc.dram_tensor("out", [QLOC, DY], F32, kind="ExternalOutput").ap()

    # collective bounce buffers (per matrix x sub-block)
    lam_in = {}
    lam_out = {}
    for mat in "mv":
        for s in range(S_SPLIT):
            lam_in[mat, s] = nc.dram_tensor(
                f"lam_in_{mat}{s}", [RSUB, DY], F32, kind="Internal").ap()
            lam_out[mat, s] = nc.dram_tensor(
                f"lam_out_{mat}{s}", [NCORES * RSUB, DY], F32,
                kind="Internal", addr_space="Shared").ap()

    with tile.TileContext(nc) as tc, ExitStack() as ctx:
        P = lambda **kw: ctx.enter_context(tc.tile_pool(**kw))
        const = P(name="const", bufs=1)
        ktp = P(name="ktp", bufs=KT_BUFS)   # [128, 1024] bf16 exp tiles
        invp = P(name="invp", bufs=3)       # inv chunks
        lstp = P(name="lstp", bufs=2)       # lambda gather stage-in
        work = P(name="work", bufs=1)
        psS = P(name="psS", bufs=2, space="PSUM")    # S matmul 2-bank pairs
        psZ = P(name="psZ", bufs=1, space="PSUM")    # z accumulators
        psA = P(name="psA", bufs=1, space="PSUM")    # stage-A accumulator
        psT = P(name="psT", bufs=1, space="PSUM")    # transposes

        ident = const.tile([128, 128], F32, tag="ident")
        masks.make_identity(nc, ident[:])

        # ------- setup loads: X-norm chain first (it gates all S matmuls) ---
        XmT_sb = const.tile([DM + 1, NX], BF16, tag="XmT_sb")
        nc.sync.dma_start(XmT_sb[0:DM, :], XmT)
        ones_sb = const.tile([128, 1], BF16, tag="ones_sb")
        nc.vector.memset(ones_sb[:], 1.0)

        # --------- X norm rows (-|X|^2/2 into XT_sb row DM/DV) --------------
        # Square X.T on DVE, then a ones-vector matmul reduces over the
        # feature partitions; tile_position lands the [1, 512] result rows
        # directly on the XT norm-row partition (DM=64 / DV=96), so a plain
        # same-base copy finishes the job.
        def x_norm_row(XT_sb, dfeat, use_act):
            sq = work.tile([DV, NX], BF16, tag="xsq", name=f"xsq{dfeat}")
            nc.vector.tensor_mul(sq[0:dfeat, :], XT_sb[0:dfeat, :],
                                 XT_sb[0:dfeat, :])
            for ch in range(NX // 512):
                ps = psS.tile([128, 1024], F32, tag="ps", name=f"xn{dfeat}_{ch}")
                nc.tensor.matmul(
                    ps[dfeat:dfeat + 1, 0:512],
                    ones_sb[0:dfeat, :],
                    sq[0:dfeat, ch * 512:(ch + 1) * 512],
                    start=True, stop=True,
                    tile_position=(0, dfeat))
                dst = XT_sb[dfeat:dfeat + 1, ch * 512:(ch + 1) * 512]
                if use_act:
                    nc.scalar.mul(dst, ps[dfeat:dfeat + 1, 0:512], -0.5)
                else:
                    nc.vector.tensor_scalar_mul(
                        dst, ps[dfeat:dfeat + 1, 0:512], -0.5)

        x_norm_row(XmT_sb, DM, False)

        Zm_sb = const.tile([128, NXT * DY], BF16, tag="Zm_sb")
        nc.sync.dma_start(Zm_sb[:], Zm)

        # ---------------- query-side assembly ----------------
        # DVE tensor ops need all operands at the same start partition, so
        # stage each transposed query block at its destination partition.
        xmT = const.tile([DM + 1, QLOC], BF16, tag="xmT")
        xvT = const.tile([DV + 1, QLOC], BF16, tag="xvT")
        ym_st = const.tile([DM, QLOC], BF16, tag="ym_st")
        yv_st = const.tile([DM, QLOC], BF16, tag="yv_st")
        yef_st = const.tile([DM, QLOC], BF16, tag="yef_st")
        nc.sync.dma_start(xmT[0:DX, :], xmuT)
        nc.sync.dma_start(xvT[0:DX, :], xmuT)
        nc.sync.dma_start(ym_st[DX:DM, :], ymT)
        nc.sync.dma_start(yv_st[DX:DM, :], yvT)
        nc.sync.dma_start(yef_st[DX:DM, :], yefT)
        nc.vector.tensor_add(xmT[DX:DM, :], ym_st[DX:DM, :], yv_st[DX:DM, :])
        nc.vector.memset(xmT[DM:DM + 1, :], 1.0)
        nc.vector.tensor_scalar_mul(xvT[DX:DM, :], yef_st[DX:DM, :], 0.01)
        # realign (ym+yv).T to partitions 64:96 via SBUF->SBUF DMA
        nc.gpsimd.dma_start(xvT[DM:DV, :], xmT[DX:DM, :])
        nc.vector.memset(xvT[DV:DV + 1, :], 1.0)

        # naturals: ymv (final additive term) + query norm scales
        # (deferred: only needed by the phase tails, emitted mid-schedule)
        qpk_sb = const.tile([128, 4 * QT * DY], F32, tag="qpk_sb")
        ymv_sb = const.tile([128, QT * DY], F32, tag="ymv_sb")
        e_m = const.tile([128, QT], F32, tag="e_m")
        e_v = const.tile([128, QT], F32, tag="e_v")

        def q_norm_setup():
            nc.sync.dma_start(qpk_sb[:], qpk)
            nc.vector.tensor_add(ymv_sb[:], qpk_sb[:, 2 * QT * DY:3 * QT * DY],
                             qpk_sb[:, 3 * QT * DY:4 * QT * DY])
            qsq = work.tile([128, 2 * QT * DY], F32, tag="qsq")
            nc.vector.tensor_mul(qsq[:, 0:QT * DY], qpk_sb[:, 0:QT * DY],
                                 qpk_sb[:, 0:QT * DY])          # x_mu^2
            nc.vector.tensor_mul(qsq[:, QT * DY:],
                                 qpk_sb[:, QT * DY:2 * QT * DY],
                                 qpk_sb[:, QT * DY:2 * QT * DY])  # yef^2
            ymvsq = work.tile([128, QT * DY], F32, tag="ymvsq")
            nc.vector.tensor_mul(ymvsq[:], ymv_sb[:], ymv_sb[:])
            r_xmu = work.tile([128, QT], F32, tag="r_xmu")
            r_yef = work.tile([128, QT], F32, tag="r_yef")
            r_ymv = work.tile([128, QT], F32, tag="r_ymv")
            RD = lambda dst, src: nc.vector.tensor_reduce(
                dst[:], src.rearrange("p (j d) -> p j d", d=DY),
                mybir.AxisListType.X, mybir.AluOpType.add)
            RD(r_xmu, qsq[:, 0:QT * DY])
            RD(r_yef, qsq[:, QT * DY:])
            RD(r_ymv, ymvsq[:])
            nc.vector.tensor_add(e_m[:], r_xmu[:], r_ymv[:])
            nc.vector.tensor_scalar_mul(r_yef[:], r_yef[:], 1.0e-4)
            nc.vector.tensor_add(e_v[:], e_m[:], r_yef[:])
            nc.scalar.activation(e_m[:], e_m[:], EXP, scale=-1.0 / 128.0)
            nc.scalar.activation(e_v[:], e_v[:], EXP, scale=-1.0 / 128.0)

        # ---------------- stage A: Lambda sub-blocks + sub-gathers ----------
        def stage_a_sub(inv_dram, Z_sb, mat, s):
            pa = psA.tile([128, RSUB], F32, tag="pa", name=f"pa_{mat}{s}")
            for q in range(NCH):
                chunk = invp.tile([128, KTC * RSUB], BF16, tag="invchunk")
                nc.sync.dma_start(chunk[:], inv_dram[s, q])
                for k8 in range(KTC):
                    kt = q * KTC + k8
                    g = kt % 4
                    nc.tensor.matmul(
                        pa[32 * g:32 * (g + 1), :],
                        Z_sb[:, kt * DY:(kt + 1) * DY],
                        chunk[:, k8 * RSUB:(k8 + 1) * RSUB],
                        start=(kt < 4), stop=(kt >= NXT - 4),
                        tile_position=(0, 32 * g))
            # transpose col-group partials so the group sum becomes a legal
            # same-base free-dim add: pa [4g*32d, r] -> T [r, 4g*32d]
            paS = work.tile([128, RSUB], F32, tag="paS")
            nc.vector.tensor_copy(paS[:], pa[:])
            lam_nat = work.tile([128, ISUB * DY], F32, tag="lam_nat")
            for i in range(ISUB):
                pt = psT.tile([128, 128], F32, tag="pt")
                nc.tensor.transpose(pt[:], paS[:, i * 128:(i + 1) * 128],
                                    ident[:])
                t0 = work.tile([128, 2 * DY], F32, tag="lam_t0")
                nc.vector.tensor_copy(t0[:], pt[:, 0:64])
                nc.vector.tensor_add(t0[:], t0[:], pt[:, 64:128])
                nc.vector.tensor_add(lam_nat[:, i * DY:(i + 1) * DY],
                                     t0[:, 0:DY], t0[:, DY:2 * DY])
            nc.gpsimd.dma_start(
                lam_in[mat, s].rearrange("(i p) d -> p i d", p=128),
                lam_nat[:].rearrange("p (i d) -> p i d", d=DY))
            nc.gpsimd.collective_compute(
                "AllGather", mybir.AluOpType.bypass,
                replica_groups=[list(range(NCORES))],
                ins=[lam_in[mat, s].opt()], outs=[lam_out[mat, s].opt()])

        def lam_stage_in(lam_sb, mat, s):
            lst = lstp.tile([128, NCORES * ISUB * DY], F32, tag="lst")
            nc.gpsimd.dma_start(
                lst[:].rearrange("p (c i d) -> p c i d", c=NCORES, i=ISUB),
                lam_out[mat, s].rearrange("(c i p) d -> p c i d",
                                          i=ISUB, p=128))
            # scatter to lam_sb columns: j = 8c + ISUB*s + i
            dst = lam_sb[:].rearrange("p (c r) -> p c r", c=NCORES)[
                :, :, ISUB * DY * s:ISUB * DY * (s + 1)]
            nc.vector.tensor_copy(
                dst, lst[:].rearrange("p (c r) -> p c r", c=NCORES))

        lam_m_sb = const.tile([128, NXT * DY], BF16, tag="lam_m_sb")
        lam_v_sb = const.tile([128, NXT * DY], BF16, tag="lam_v_sb")
        XvT_sb = const.tile([DV + 1, NX], BF16, tag="XvT_sb")
        Zv_sb = const.tile([128, NXT * DY], BF16, tag="Zv_sb")

        # ---------------- RBF phase machinery ----------------
        def phase_mk(XT_sb, dfeat, xT_sb, lam_sb, combine, name):
            """Returns (emit_stage(st), emit_z(st), tail()) closures."""
            jl = _jlist(0)
            kts = {}
            pz = [psZ.tile([128, 512], F32, tag=f"pz{qc}", name=f"pz{qc}_{name}")
                  for qc in range(2)]

            def emit_stage(st):
                jset = jl[JPS * st:JPS * (st + 1)]
                for jj, j in enumerate(jset):
                    ps = psS.tile([128, 1024], F32, tag="ps")
                    for qc in range(2):
                        nc.tensor.matmul(
                            ps[:, qc * 512:(qc + 1) * 512],
                            XT_sb[0:dfeat + 1, j * 128:(j + 1) * 128],
                            xT_sb[0:dfeat + 1, qc * 512:(qc + 1) * 512],
                            start=True, stop=True)
                    kt = ktp.tile([128, QLOC], BF16, tag="kt")
                    nc.scalar.activation(kt[:], ps[:], EXP, scale=1.0 / 64.0)
                    kts[st, jj] = kt

            def emit_z(st):
                jset = jl[JPS * st:JPS * (st + 1)]
                for qc in range(2):
                    for g, j in enumerate(jset):
                        nc.tensor.matmul(
                            pz[qc][32 * g:32 * (g + 1), :],
                            lam_sb[:, j * DY:(j + 1) * DY],
                            kts[st, g][:, qc * 512:(qc + 1) * 512],
                            start=(st == 0), stop=(st == NST - 1),
                            tile_position=(0, 32 * g))

            def tail():
                # pz [4g*32d, q] -> transpose 128-col slices -> [q, 4g*32d],
                # then sum groups along the free dim (natural q layout).
                for qc in range(2):
                    pzS = work.tile([128, 512], F32, tag="pzS")
                    nc.vector.tensor_copy(pzS[:], pz[qc][:])
                    for qq in range(4):
                        jq = 4 * qc + qq
                        pt = psT.tile([128, 128], F32, tag="pt")
                        nc.tensor.transpose(
                            pt[:], pzS[:, qq * 128:(qq + 1) * 128], ident[:])
                        t0 = work.tile([128, 2 * DY], F32, tag="z_t0")
                        nc.vector.tensor_copy(t0[:], pt[:, 0:64])
                        nc.vector.tensor_add(t0[:], t0[:], pt[:, 64:128])
                        combine(jq, t0[:, 0:DY], t0[:, DY:2 * DY])

            return emit_stage, emit_z, tail

        zM_nat = const.tile([128, QT * DY], F32, tag="zM_nat")

        def combine_mean(jq, t0, t1):
            sl = slice(jq * DY, (jq + 1) * DY)
            nc.vector.tensor_add(zM_nat[:, sl], t0, t1)
            nc.vector.tensor_scalar_mul(zM_nat[:, sl], zM_nat[:, sl],
                                        e_m[:, jq:jq + 1])

        out_sb = const.tile([128, QT * DY], F32, tag="out_sb")

        def combine_var(jq, t0, t1):
            sl = slice(jq * DY, (jq + 1) * DY)
            nc.vector.tensor_add(out_sb[:, sl], t0, t1)
            nc.vector.tensor_scalar_mul(out_sb[:, sl], out_sb[:, sl],
                                        e_v[:, jq:jq + 1])
            nc.vector.tensor_add(out_sb[:, sl], out_sb[:, sl], zM_nat[:, sl])
            nc.vector.tensor_add(out_sb[:, sl], out_sb[:, sl], ymv_sb[:, sl])

        ph_m = phase_mk(XmT_sb, DM, xmT, lam_m_sb, combine_mean, "m")
        ph_v = phase_mk(XvT_sb, DV, xvT, lam_v_sb, combine_var, "v")

        # ---------------- emission schedule ----------------
        def phase_emit(ph, st):
            emit_stage, emit_z, _ = ph
            emit_stage(st)
            if st - Z_LAG >= 0:
                emit_z(st - Z_LAG)

        stage_a_sub(invm, Zm_sb, "m", 0)
        lam_stage_in(lam_m_sb, "m", 0)
        for st in range(0, 4):
            phase_emit(ph_m, st)
        stage_a_sub(invm, Zm_sb, "m", 1)
        lam_stage_in(lam_m_sb, "m", 1)
        for st in range(4, 8):
            phase_emit(ph_m, st)

        # var-side loads + deferred query-norm setup
        nc.sync.dma_start(XvT_sb[0:DV, :], XvT)
        nc.sync.dma_start(Zv_sb[:], Zv)
        q_norm_setup()

        for st in range(8, 12):
            phase_emit(ph_m, st)
        x_norm_row(XvT_sb, DV, False)
        for st in range(12, 16):
            phase_emit(ph_m, st)
        # trailing z of phase m
        for st in range(NST - Z_LAG, NST):
            ph_m[1](st)
        ph_m[2]()  # tail m

        # phase v S/exp can start while inv v still streams; its stage-A
        # sub-blocks are interleaved at DMA-availability-aligned points.
        # NOTE: z_v for stages 0..1 must come after lam_stage_in(v, 0) in
        # program order or they read an unwritten lam_v_sb.
        for st in range(0, 4):
            ph_v[0](st)
        stage_a_sub(invv, Zv_sb, "v", 0)
        lam_stage_in(lam_v_sb, "v", 0)
        for st in range(0, 2):
            ph_v[1](st)
        for st in range(4, 8):
            phase_emit(ph_v, st)
        stage_a_sub(invv, Zv_sb, "v", 1)
        lam_stage_in(lam_v_sb, "v", 1)
        for st in range(8, 16):
            phase_emit(ph_v, st)
        for st in range(NST - Z_LAG, NST):
            ph_v[1](st)
        ph_v[2]()  # tail v (writes out_sb)

        nc.gpsimd.dma_start(out.rearrange("(t p) d -> p t d", p=128),
                            out_sb[:].rearrange("p (t d) -> p t d", d=DY))

    nc.compile()
    return nc


def get_nc():
    global _CACHED_NC
    if _CACHED_NC is None:
        _CACHED_NC = _build_nc()
    return _CACHED_NC


def _host_prep(x_mu, y_eta, y_mean, y_var, X_mean, X_var, Z_mean, Z_var,
               kXXmean_inv, kXXvar_inv):
    """Host prep: transposes / slicing / packing / bf16 casts only."""
    BF = ml_dtypes.bfloat16
    C = np.ascontiguousarray

    def pack_inv(inv):
        # per-core [S_SPLIT, NCH, 128, KTC*RSUB] slabs of inv.T in bf16
        invT = C(inv.T).astype(BF)
        packs = []
        for c in range(NCORES):
            A = invT[:, c * RLOC:(c + 1) * RLOC]
            A = A.reshape(NXT, 128, S_SPLIT, RSUB).transpose(2, 0, 1, 3)
            A = A.reshape(S_SPLIT, NCH, KTC, 128, RSUB).transpose(0, 1, 3, 2, 4)
            packs.append(C(A.reshape(S_SPLIT, NCH, 128, KTC * RSUB)))
        return packs

    invm_p = pack_inv(kXXmean_inv)
    invv_p = pack_inv(kXXvar_inv)

    XmT = C(X_mean.T).astype(BF)
    XvT = C(X_var.T).astype(BF)
    Zm = C(Z_mean.reshape(NXT, 128, DY).transpose(1, 0, 2).reshape(128, -1)
           ).astype(BF)
    Zv = C(Z_var.reshape(NXT, 128, DY).transpose(1, 0, 2).reshape(128, -1)
           ).astype(BF)
    yef = y_eta[::-1]

    in_maps = []
    for c in range(NCORES):
        q = slice(c * QLOC, (c + 1) * QLOC)
        qpk = np.stack([x_mu[q], yef[q], y_mean[q], y_var[q]])  # [4,1024,32]
        qpk = qpk.reshape(4, QT, 128, DY).transpose(2, 0, 1, 3)
        in_maps.append({
            "invm": invm_p[c], "invv": invv_p[c],
            "XmT": XmT, "XvT": XvT, "Zm": Zm, "Zv": Zv,
            "xmuT": C(x_mu[q].T).astype(BF),
            "yefT": C(yef[q].T).astype(BF),
            "ymT": C(y_mean[q].T).astype(BF),
            "yvT": C(y_var[q].T).astype(BF),
            "qpk": C(qpk.reshape(128, -1)).astype(np.float32),
        })
    return in_maps


def kernel(x_mu, y_eta, y_mean, y_var, X_mean, X_var, Z_mean, Z_var,
           kXXmean_inv, kXXvar_inv, _trace=False, _tmpdir=None):
    nc = get_nc()
    in_maps = _host_prep(x_mu, y_eta, y_mean, y_var, X_mean, X_var,
                         Z_mean, Z_var, kXXmean_inv, kXXvar_inv)
    res = run_bass_kernel_spmd(nc, in_maps, core_ids=list(range(NCORES)),
                               trace=_trace, tmpdir=_tmpdir)
    out = np.concatenate([res.results[c]["out"] for c in range(NCORES)], axis=0)
    if _trace:
        kernel._last_results = res
    return out
